# revision 1
# baseline (speedup 1.0000x reference)
"""MoE-routed DIAYN discriminator kernel for 8 Trainium2 NeuronCores.

Reference semantics: x = concat([graph, state, next_state], -1); for each
row, run the 3-layer MLP of the LAST factor i<NF with graph[:, i]==1
(rows with no active factor output 0). The dense reference computes all
NF expert MLPs for every row; we instead route each row to exactly one
expert on the host, pack rows into 8 SPMD shards, and run one dense
per-expert MLP stream per core.

Sharding: rows are grouped by expert into BLK-row blocks. Every core
executes the same static "profile" of G runs (run g = prof[g] blocks);
each run uses one weight set, supplied per-core as data. A small host-side
search picks (G, prof) and an assignment of runs -> experts that covers
the actual per-expert block counts with minimal padding + weight traffic.

Device kernel (per run, per block, activations kept transposed [feat, row]):
  h1 = relu(W1^T x + b1); h2 = relu(W2^T h1 + b2); out = W3^T h2 + b3
matmuls run as fp32 bitcast to float32r (full-rate fp32 on the PE).
"""

import numpy as np

import concourse.bass as bass
import concourse.mybir as mybir
from concourse import bacc
from concourse.tile import TileContext
from concourse.bass_utils import run_bass_kernel_spmd

NCORES = 8
BLK = 272  # rows per matmul block; >=256 (f32r full rate), <=512 (PSUM bank)

F32 = mybir.dt.float32
F32R = mybir.dt.float32r

# Rough per-core cost weights for the plan search (ns).
_COST_BLOCK = int(152 * (BLK / 2.4 + 3))  # PE ns per block (152 matmuls)
_COST_RUN = 12_000  # partially-exposed weight-set DMA per extra run

_program_cache = {}


# ---------------------------------------------------------------- planning
def _compositions(total, parts):
    """Non-increasing positive integer compositions of `total` into `parts`."""
    if parts == 1:
        yield (total,)
        return
    for first in range((total + parts - 1) // parts, total - parts + 2):
        for rest in _compositions(total - first, parts - 1):
            if rest[0] <= first:
                yield (first,) + rest


def _try_assign(demands, prof):
    """Greedy cover of per-expert block demands by the 8x-replicated profile.

    demands: list of (n_blocks, expert) sorted desc. Returns dict
    run_size -> list of experts (8 entries per profile slot of that size,
    padding slots filled with the largest expert) or None if infeasible.
    """
    runs = sorted([t for t in prof for _ in range(NCORES)], reverse=True)
    used = []  # (size, expert)
    for n, e in demands:
        rem = n
        while rem > 0:
            if not runs:
                return None
            # largest run <= rem, else smallest run (minimal overshoot)
            pick = None
            for i, s in enumerate(runs):
                if s <= rem:
                    pick = i
                    break
            if pick is None:
                pick = len(runs) - 1
            s = runs.pop(pick)
            used.append((s, e))
            rem -= s
    pad_expert = demands[0][1]
    for s in runs:
        used.append((s, pad_expert))
    by_size = {}
    for s, e in used:
        by_size.setdefault(s, []).append(e)
    return by_size


def _make_plan(nblk):
    """nblk: per-expert block counts. Returns (prof, expert_of[core][g])."""
    demands = sorted(
        [(n, e) for e, n in enumerate(nblk) if n > 0], reverse=True
    )
    total = sum(n for n, _ in demands)
    mincap = (total + NCORES - 1) // NCORES
    best = None
    for G in range(1, 9):
        for cap in range(mincap, mincap + 6):
            for prof in _compositions(cap, G):
                a = _try_assign(demands, prof)
                if a is None:
                    continue
                cost = cap * _COST_BLOCK + G * _COST_RUN
                if best is None or cost < best[0]:
                    best = (cost, prof, a)
    assert best is not None, "no feasible run plan found"
    _, prof, by_size = best
    queues = {s: list(es) for s, es in by_size.items()}
    expert_of = [[None] * len(prof) for _ in range(NCORES)]
    for g, s in enumerate(prof):
        for core in range(NCORES):
            expert_of[core][g] = queues[s].pop(0)
    return list(prof), expert_of


# ---------------------------------------------------------------- device
def _build_program(prof, KO1, KO2, H, C, blk):
    """Build + compile the SPMD Bass program for a run profile."""
    key = (tuple(prof), KO1, KO2, H, C, blk)
    if key in _program_cache:
        return _program_cache[key]

    G = len(prof)
    NB = sum(prof)
    INP = KO1 * 128
    M1 = H // 128
    relu = mybir.ActivationFunctionType.Relu
    ident = mybir.ActivationFunctionType.Identity

    nc = bacc.Bacc("TRN2", target_bir_lowering=False, debug=False,
                   num_devices=NCORES)
    x_d = nc.dram_tensor("xb", [NB, 128, KO1, blk], F32R, kind="ExternalInput").ap()
    w1_d = nc.dram_tensor("w1", [G, 128, KO1, H], F32R, kind="ExternalInput").ap()
    w2_d = nc.dram_tensor("w2", [G, 128, KO2, H], F32R, kind="ExternalInput").ap()
    w3_d = nc.dram_tensor("w3", [G, 128, KO2, C], F32R, kind="ExternalInput").ap()
    b1_d = nc.dram_tensor("b1", [G, H], F32, kind="ExternalInput").ap()
    b2_d = nc.dram_tensor("b2", [G, H], F32, kind="ExternalInput").ap()
    b3_d = nc.dram_tensor("b3", [G, C], F32, kind="ExternalInput").ap()
    out_d = nc.dram_tensor("outb", [NB, C, blk], F32, kind="ExternalOutput").ap()

    runs = []
    for g, T in enumerate(prof):
        runs += [g] * T

    with TileContext(nc) as tc:
        with (
            tc.tile_pool(name="w", bufs=2) as wpool,
            tc.tile_pool(name="x", bufs=2) as xpool,
            tc.tile_pool(name="h1", bufs=3) as h1pool,
            tc.tile_pool(name="h2", bufs=1) as h2pool,
            tc.tile_pool(name="o", bufs=2) as opool,
            tc.tile_pool(name="ps", bufs=8, space="PSUM") as pspool,
        ):
            def emit_weights(g, x_first=None, x_hook=None, x_hook2=None):
                # Biases first (tiny, needed by the first relu). W1 as
                # per-k-tile chunks so block-0's k-outer L1 can consume
                # them as they arrive; W2 as halves (needed later).
                w1ch = []
                b1sb = b2sb = b3sb = None
                for k in range(KO1):
                    if x_first is not None:
                        nc.sync.dma_start(x_first[0][:, k, :],
                                          x_first[1][:, k, :])
                    wt = wpool.tile([128, H], F32R, tag=f"w1k{k}")
                    nc.sync.dma_start(wt[:], w1_d[g, :, k, :])
                    w1ch.append(wt)
                    if k == 0:
                        # Biases after the first chunk pair (PE can start)
                        # but well before the first relu needs them.
                        b1sb = wpool.tile([128, M1], F32, tag="b1")
                        nc.sync.dma_start(
                            b1sb[:],
                            b1_d[g].rearrange("(m p) -> p m", p=128))
                        b2sb = wpool.tile([128, M1], F32, tag="b2")
                        nc.sync.dma_start(
                            b2sb[:],
                            b2_d[g].rearrange("(m p) -> p m", p=128))
                        b3sb = wpool.tile([C, 1], F32, tag="b3")
                        nc.sync.dma_start(b3sb[:], b3_d[g][:, None])
                if x_hook is not None:
                    x_hook()
                KH2 = KO2 // 2
                w2a = wpool.tile([128, KH2, H], F32R, tag="w2a")
                nc.sync.dma_start(w2a[:], w2_d[g, :, :KH2, :])
                if x_hook2 is not None:
                    x_hook2()
                w2b = wpool.tile([128, KO2 - KH2, H], F32R, tag="w2b")
                nc.sync.dma_start(w2b[:], w2_d[g, :, KH2:, :])
                w3sb = wpool.tile([128, KO2, C], F32R, tag="w3")
                nc.sync.dma_start(w3sb[:], w3_d[g])

                def w2(k):
                    return w2a[:, k, :] if k < KH2 else w2b[:, k - KH2, :]

                return dict(w1=lambda k: w1ch[k], w2=w2, w3=w3sb,
                            b1=b1sb, b2=b2sb, b3=b3sb)

            def emit_x(b):
                # x blocks ride the second HWDGE ring (scalar), parallel
                # to the weight stream on sync.
                xsb = xpool.tile([128, KO1, blk], F32R, tag="x")
                nc.scalar.dma_start(xsb[:], x_d[b])
                return xsb

            def emit_L1(W, xsb, kouter=False):
                h1sb = h1pool.tile([128, KO2, blk], F32R, tag="h1")
                if kouter:
                    # All 8 PSUM banks accumulate in parallel; each W1
                    # chunk is fully consumed on arrival (startup mode).
                    pss = [pspool.tile([128, blk], F32, tag="ps",
                                       name=f"ps_ko{m}")
                           for m in range(M1)]
                    for k in range(KO1):
                        for m in range(M1):
                            nc.tensor.matmul(
                                pss[m][:],
                                W["w1"](k)[:, m * 128:(m + 1) * 128],
                                xsb[:, k, :],
                                start=(k == 0), stop=(k == KO1 - 1))
                    for m in range(M1):
                        nc.vector.tensor_scalar(
                            h1sb[:, m, :], pss[m][:], W["b1"][:, m:m + 1],
                            0.0, mybir.AluOpType.add, mybir.AluOpType.max)
                    return h1sb
                for m in range(M1):
                    ps = pspool.tile([128, blk], F32, tag="ps")
                    for k in range(KO1):
                        nc.tensor.matmul(
                            ps[:],
                            W["w1"](k)[:, m * 128:(m + 1) * 128],
                            xsb[:, k, :],
                            start=(k == 0), stop=(k == KO1 - 1))
                    nc.vector.tensor_scalar(
                        h1sb[:, m, :], ps[:], W["b1"][:, m:m + 1], 0.0,
                        mybir.AluOpType.add, mybir.AluOpType.max)
                return h1sb

            def emit_L23(b, W, h1sb):
                h2sb = h2pool.tile([128, KO2, blk], F32R, tag="h2")
                for m in range(M1):
                    ps = pspool.tile([128, blk], F32, tag="ps")
                    for k in range(KO2):
                        nc.tensor.matmul(
                            ps[:],
                            W["w2"](k)[:, m * 128:(m + 1) * 128],
                            h1sb[:, k, :],
                            start=(k == 0), stop=(k == KO2 - 1))
                    nc.scalar.activation(
                        h2sb[:, m, :], ps[:], relu, bias=W["b2"][:, m:m + 1])
                ps3 = pspool.tile([128, blk], F32, tag="ps")
                for k in range(KO2):
                    nc.tensor.matmul(
                        ps3[:C, :],
                        W["w3"][:, k, :],
                        h2sb[:, k, :],
                        start=(k == 0), stop=(k == KO2 - 1))
                osb = opool.tile([C, blk], F32, tag="o")
                nc.scalar.activation(
                    osb[:], ps3[:C, :], ident, bias=W["b3"][:, 0:1])
                nc.gpsimd.dma_start(out_d[b], osb[:])

            # Software pipeline, depth 2: L1 of blocks b+1/b+2 are
            # emitted before L2/L3 of block b, so weight-set DMAs and
            # ACT latency never drain the PE (esp. during the initial
            # HBM-bound weight load).
            Ws = {}
            h1 = {}

            xpre = {}

            def emit_front(b):
                g = runs[b]
                if g not in Ws:
                    Ws[g] = emit_weights(g)
                h1[b] = emit_L1(Ws[g], xpre.pop(b) if b in xpre
                                else emit_x(b))

            # Startup: x0/x1 lead the scalar ring while weights
            # stream on sync; L1(0)/L1(1) are emitted before L2(0) so
            # the PE has work during the HBM-bound weight load. Steady
            # state keeps L1 two blocks ahead of L2/L3.
            def emit_x_sync(b):
                xsb = xpool.tile([128, KO1, blk], F32R, tag="x")
                nc.sync.dma_start(xsb[:], x_d[b])
                return xsb

            # Startup: everything for the first ~3 blocks rides the sync
            # ring in consumption order (x0 interleaved with W1 chunks,
            # then x1, W2a, x2, W2b); block 0's L1 runs k-outer so each
            # W1 chunk is consumed on arrival.
            g0 = runs[0]
            if prof[0] >= 3:
                xsb0 = xpool.tile([128, KO1, blk], F32R, tag="x")
                xs = {}
                def _x12():
                    xs[1] = emit_x_sync(1)
                    xs[2] = emit_x_sync(2)

                Ws[g0] = emit_weights(g0, x_first=(xsb0, x_d[0]),
                                      x_hook=_x12)
                # x3/x4 ride the idle SWDGE ring: the scalar ring's
                # issue slot is blocked behind early L2-relus right at
                # the prologue->steady transition.
                for bb in (3, 4):
                    if bb < NB:
                        xp = xpool.tile([128, KO1, blk], F32R, tag="x",
                                        name=f"xpre{bb}")
                        nc.gpsimd.dma_start(xp[:], x_d[bb])
                        xpre[bb] = xp
                h1[0] = emit_L1(Ws[g0], xsb0, kouter=True)
                h1[1] = emit_L1(Ws[g0], xs[1])
                h1[2] = emit_L1(Ws[g0], xs[2])
                emitted = 2
            elif NB > 1 and runs[1] == g0:
                xsb0 = xpool.tile([128, KO1, blk], F32R, tag="x")
                xs1 = []
                Ws[g0] = emit_weights(g0, x_first=(xsb0, x_d[0]),
                                      x_hook=lambda: xs1.append(emit_x(1)))
                h1[0] = emit_L1(Ws[g0], xsb0, kouter=True)
                h1[1] = emit_L1(Ws[g0], xs1[0])
                emitted = 1
            else:
                emit_front(0)
                emitted = 0
            for b in range(NB):
                for nxt in range(emitted + 1, min(b + 3, NB)):
                    emit_front(nxt)
                    emitted = nxt
                if b + 4 < NB and runs[b + 4] not in Ws:
                    Ws[runs[b + 4]] = emit_weights(runs[b + 4])
                emit_L23(b, Ws[runs[b]], h1.pop(b))

    nc.compile()
    _program_cache[key] = nc
    return nc


# ---------------------------------------------------------------- host
def _execute(inputs, trace=False, trace_cores=None):
    graph = np.ascontiguousarray(inputs["graph"], dtype=np.float32)
    state = np.ascontiguousarray(inputs["state"], dtype=np.float32)
    next_state = np.ascontiguousarray(inputs["next_state"], dtype=np.float32)
    W1 = np.ascontiguousarray(inputs["W1"], dtype=np.float32)
    b1 = np.ascontiguousarray(inputs["b1"], dtype=np.float32)
    W2 = np.ascontiguousarray(inputs["W2"], dtype=np.float32)
    b2 = np.ascontiguousarray(inputs["b2"], dtype=np.float32)
    W3 = np.ascontiguousarray(inputs["W3"], dtype=np.float32)
    b3 = np.ascontiguousarray(inputs["b3"], dtype=np.float32)

    B = graph.shape[0]
    NF, IN, H = W1.shape
    C = W3.shape[2]
    assert IN == graph.shape[1] + state.shape[1] + next_state.shape[1]
    assert H % 128 == 0 and C <= 128
    INP = ((IN + 127) // 128) * 128
    KO1 = INP // 128

    out_full = np.zeros((B, C), dtype=np.float32)

    # --- route: last active factor per row
    mask = graph[:, :NF] == 1.0
    active = mask.any(axis=1)
    last = (NF - 1) - np.argmax(mask[:, ::-1], axis=1)
    if not active.any():
        return (out_full, None) if trace else out_full

    rows_by_e = [np.nonzero(active & (last == e))[0] for e in range(NF)]
    nblk = [(len(r) + BLK - 1) // BLK for r in rows_by_e]
    prof, expert_of = _make_plan(nblk)
    G, NB = len(prof), sum(prof)

    # --- pack rows into per-core block slots
    # rowmap[core] : int32 [NB, BLK], original row id or -1 (pad)
    rowmap = [np.full((NB, BLK), -1, dtype=np.int64) for _ in range(NCORES)]
    off = np.cumsum([0] + prof)  # run g occupies blocks [off[g], off[g+1])
    slots_by_e = {}
    for core in range(NCORES):
        for g in range(G):
            slots_by_e.setdefault(expert_of[core][g], []).append((core, g))
    for e in range(NF):
        rows = rows_by_e[e]
        if len(rows) == 0:
            continue
        pos = 0
        for core, g in slots_by_e.get(e, []):
            cap = prof[g] * BLK
            take = min(cap, len(rows) - pos)
            if take <= 0:
                break
            flat = rowmap[core][off[g]:off[g + 1]].reshape(-1)
            flat[:take] = rows[pos:pos + take]
            pos += take
        assert pos == len(rows), f"expert {e} rows not fully packed"

    # --- build per-core inputs
    x = np.concatenate([graph, state, next_state], axis=1)  # [B, IN]
    if INP != IN:
        x = np.concatenate([x, np.zeros((B, INP - IN), np.float32)], axis=1)
    xpad = np.concatenate([x, np.zeros((1, INP), np.float32)], axis=0)
    W1p = np.zeros((NF, INP, H), np.float32)
    W1p[:, :IN] = W1

    # Partition-major device layouts: [.., 128, KO, free] so every DMA
    # line is one contiguous 10-20KB run per partition.
    KO2 = H // 128
    W1pm = np.ascontiguousarray(
        W1p.reshape(NF, KO1, 128, H).transpose(0, 2, 1, 3))
    W2pm = np.ascontiguousarray(
        W2.reshape(NF, KO2, 128, H).transpose(0, 2, 1, 3))
    W3pm = np.ascontiguousarray(
        W3.reshape(NF, KO2, 128, C).transpose(0, 2, 1, 3))
    in_maps = []
    for core in range(NCORES):
        xb = xpad[rowmap[core].reshape(-1)]  # [NB*BLK, INP]; -1 -> zero row
        xb = np.ascontiguousarray(
            xb.reshape(NB, BLK, KO1, 128).transpose(0, 3, 2, 1))
        es = expert_of[core]
        in_maps.append({
            "xb": xb,
            "w1": W1pm[es],
            "w2": W2pm[es],
            "w3": W3pm[es],
            "b1": np.ascontiguousarray(b1[es]),
            "b2": np.ascontiguousarray(b2[es]),
            "b3": np.ascontiguousarray(b3[es]),
        })

    nc = _build_program(prof, KO1, KO2, H, C, BLK)
    kwargs = {}
    if trace:
        kwargs = dict(trace=True,
                      trace_cores=trace_cores or list(range(NCORES)))
    res = run_bass_kernel_spmd(nc, in_maps, list(range(NCORES)), **kwargs)

    # --- scatter back
    for core in range(NCORES):
        ob = np.asarray(res.results[core]["outb"])  # [NB, C, BLK]
        rows = ob.transpose(0, 2, 1).reshape(NB * BLK, C)
        ids = rowmap[core].reshape(-1)
        valid = ids >= 0
        out_full[ids[valid]] = rows[valid]

    return (out_full, res) if trace else out_full


def kernel(**inputs):
    return _execute(inputs)



# revision 3
# speedup vs baseline: 1.1182x; 1.1182x over previous
"""MoE-routed DIAYN discriminator kernel for 8 Trainium2 NeuronCores.

Reference semantics: x = concat([graph, state, next_state], -1); for each
row, run the 3-layer MLP of the LAST factor i<NF with graph[:, i]==1
(rows with no active factor output 0). The dense reference computes all
NF expert MLPs for every row; we instead route each row to exactly one
expert on the host, pack rows into 8 SPMD shards, and run one dense
per-expert MLP stream per core.

Sharding: rows are grouped by expert into BLK-row blocks. Every core
executes the same static "profile" of G runs (run g = prof[g] blocks);
each run uses one weight set, supplied per-core as data. A small host-side
search picks (G, prof) and an assignment of runs -> experts that covers
the actual per-expert block counts with minimal padding + weight traffic.

Device kernel (per run, per block, activations kept transposed [feat, row]):
  h1 = relu(W1^T x + b1); h2 = relu(W2^T h1 + b2); out = W3^T h2 + b3
matmuls run as fp32 bitcast to float32r (full-rate fp32 on the PE).
"""

import numpy as np
from ml_dtypes import bfloat16

import concourse.bass as bass
import concourse.mybir as mybir
from concourse import bacc
from concourse.tile import TileContext
from concourse.bass_utils import run_bass_kernel_spmd

NCORES = 8
BLK = 272  # rows per matmul block; <=512 (PSUM bank)

F32 = mybir.dt.float32
BF16 = mybir.dt.bfloat16

# Rough per-core cost weights for the plan search (ns).
_COST_BLOCK = int(152 * (BLK / 2.4 + 3))  # PE ns per block (152 matmuls)
_COST_RUN = 12_000  # partially-exposed weight-set DMA per extra run

_program_cache = {}


# ---------------------------------------------------------------- planning
def _compositions(total, parts):
    """Non-increasing positive integer compositions of `total` into `parts`."""
    if parts == 1:
        yield (total,)
        return
    for first in range((total + parts - 1) // parts, total - parts + 2):
        for rest in _compositions(total - first, parts - 1):
            if rest[0] <= first:
                yield (first,) + rest


def _try_assign(demands, prof):
    """Greedy cover of per-expert block demands by the 8x-replicated profile.

    demands: list of (n_blocks, expert) sorted desc. Returns dict
    run_size -> list of experts (8 entries per profile slot of that size,
    padding slots filled with the largest expert) or None if infeasible.
    """
    runs = sorted([t for t in prof for _ in range(NCORES)], reverse=True)
    used = []  # (size, expert)
    for n, e in demands:
        rem = n
        while rem > 0:
            if not runs:
                return None
            # largest run <= rem, else smallest run (minimal overshoot)
            pick = None
            for i, s in enumerate(runs):
                if s <= rem:
                    pick = i
                    break
            if pick is None:
                pick = len(runs) - 1
            s = runs.pop(pick)
            used.append((s, e))
            rem -= s
    pad_expert = demands[0][1]
    for s in runs:
        used.append((s, pad_expert))
    by_size = {}
    for s, e in used:
        by_size.setdefault(s, []).append(e)
    return by_size


def _make_plan(nblk):
    """nblk: per-expert block counts. Returns (prof, expert_of[core][g])."""
    demands = sorted(
        [(n, e) for e, n in enumerate(nblk) if n > 0], reverse=True
    )
    total = sum(n for n, _ in demands)
    mincap = (total + NCORES - 1) // NCORES
    best = None
    for G in range(1, 9):
        for cap in range(mincap, mincap + 6):
            for prof in _compositions(cap, G):
                a = _try_assign(demands, prof)
                if a is None:
                    continue
                cost = cap * _COST_BLOCK + G * _COST_RUN
                if best is None or cost < best[0]:
                    best = (cost, prof, a)
    assert best is not None, "no feasible run plan found"
    _, prof, by_size = best
    queues = {s: list(es) for s, es in by_size.items()}
    expert_of = [[None] * len(prof) for _ in range(NCORES)]
    for g, s in enumerate(prof):
        for core in range(NCORES):
            expert_of[core][g] = queues[s].pop(0)
    return list(prof), expert_of


# ---------------------------------------------------------------- device
def _build_program(prof, KO1, KO2, H, C, blk):
    """Build + compile the SPMD Bass program for a run profile."""
    key = (tuple(prof), KO1, KO2, H, C, blk)
    if key in _program_cache:
        return _program_cache[key]

    G = len(prof)
    NB = sum(prof)
    INP = KO1 * 128
    M1 = H // 128
    relu = mybir.ActivationFunctionType.Relu
    ident = mybir.ActivationFunctionType.Identity

    nc = bacc.Bacc("TRN2", target_bir_lowering=False, debug=False,
                   num_devices=NCORES)
    x_d = nc.dram_tensor("xb", [NB, 128, KO1, blk], BF16, kind="ExternalInput").ap()
    w1_d = nc.dram_tensor("w1", [G, 128, KO1, H], BF16, kind="ExternalInput").ap()
    w2_d = nc.dram_tensor("w2", [G, 128, KO2, H], BF16, kind="ExternalInput").ap()
    w3_d = nc.dram_tensor("w3", [G, 128, KO2, C], BF16, kind="ExternalInput").ap()
    b1_d = nc.dram_tensor("b1", [G, H], F32, kind="ExternalInput").ap()
    b2_d = nc.dram_tensor("b2", [G, H], F32, kind="ExternalInput").ap()
    b3_d = nc.dram_tensor("b3", [G, C], F32, kind="ExternalInput").ap()
    out_d = nc.dram_tensor("outb", [NB, C, blk], F32, kind="ExternalOutput").ap()

    runs = []
    for g, T in enumerate(prof):
        runs += [g] * T

    with TileContext(nc) as tc:
        with (
            tc.tile_pool(name="w", bufs=2) as wpool,
            tc.tile_pool(name="x", bufs=2) as xpool,
            tc.tile_pool(name="h1", bufs=3) as h1pool,
            tc.tile_pool(name="h2", bufs=1) as h2pool,
            tc.tile_pool(name="o", bufs=2) as opool,
            tc.tile_pool(name="ps", bufs=8, space="PSUM") as pspool,
        ):
            def emit_weights(g, x_first=None, x_hook=None, x_hook2=None):
                # Biases first (tiny, needed by the first relu). W1 as
                # per-k-tile chunks so block-0's k-outer L1 can consume
                # them as they arrive; W2 as halves (needed later).
                w1ch = []
                b1sb = b2sb = b3sb = None
                for k in range(KO1):
                    if x_first is not None:
                        nc.sync.dma_start(x_first[0][:, k, :],
                                          x_first[1][:, k, :])
                    wt = wpool.tile([128, H], BF16, tag=f"w1k{k}")
                    nc.sync.dma_start(wt[:], w1_d[g, :, k, :])
                    w1ch.append(wt)
                    if k == 0:
                        # Biases after the first chunk pair (PE can start)
                        # but well before the first relu needs them.
                        b1sb = wpool.tile([128, M1], F32, tag="b1")
                        nc.sync.dma_start(
                            b1sb[:],
                            b1_d[g].rearrange("(m p) -> p m", p=128))
                        b2sb = wpool.tile([128, M1], F32, tag="b2")
                        nc.sync.dma_start(
                            b2sb[:],
                            b2_d[g].rearrange("(m p) -> p m", p=128))
                        b3sb = wpool.tile([C, 1], F32, tag="b3")
                        nc.sync.dma_start(b3sb[:], b3_d[g][:, None])
                if x_hook is not None:
                    x_hook()
                KH2 = KO2 // 2
                w2a = wpool.tile([128, KH2, H], BF16, tag="w2a")
                nc.sync.dma_start(w2a[:], w2_d[g, :, :KH2, :])
                if x_hook2 is not None:
                    x_hook2()
                w2b = wpool.tile([128, KO2 - KH2, H], BF16, tag="w2b")
                nc.sync.dma_start(w2b[:], w2_d[g, :, KH2:, :])
                w3sb = wpool.tile([128, KO2, C], BF16, tag="w3")
                nc.sync.dma_start(w3sb[:], w3_d[g])

                def w2(k):
                    return w2a[:, k, :] if k < KH2 else w2b[:, k - KH2, :]

                return dict(w1=lambda k: w1ch[k], w2=w2, w3=w3sb,
                            b1=b1sb, b2=b2sb, b3=b3sb)

            def emit_x(b):
                # x blocks ride the second HWDGE ring (scalar), parallel
                # to the weight stream on sync.
                xsb = xpool.tile([128, KO1, blk], BF16, tag="x")
                nc.scalar.dma_start(xsb[:], x_d[b])
                return xsb

            def emit_L1(W, xsb, kouter=False):
                h1sb = h1pool.tile([128, KO2, blk], BF16, tag="h1")
                if kouter:
                    # All 8 PSUM banks accumulate in parallel; each W1
                    # chunk is fully consumed on arrival (startup mode).
                    pss = [pspool.tile([128, blk], F32, tag="ps",
                                       name=f"ps_ko{m}")
                           for m in range(M1)]
                    for k in range(KO1):
                        for m in range(M1):
                            nc.tensor.matmul(
                                pss[m][:],
                                W["w1"](k)[:, m * 128:(m + 1) * 128],
                                xsb[:, k, :],
                                start=(k == 0), stop=(k == KO1 - 1))
                    for m in range(M1):
                        nc.vector.tensor_scalar(
                            h1sb[:, m, :], pss[m][:], W["b1"][:, m:m + 1],
                            0.0, mybir.AluOpType.add, mybir.AluOpType.max)
                    return h1sb
                for m in range(M1):
                    ps = pspool.tile([128, blk], F32, tag="ps")
                    for k in range(KO1):
                        nc.tensor.matmul(
                            ps[:],
                            W["w1"](k)[:, m * 128:(m + 1) * 128],
                            xsb[:, k, :],
                            start=(k == 0), stop=(k == KO1 - 1))
                    nc.vector.tensor_scalar(
                        h1sb[:, m, :], ps[:], W["b1"][:, m:m + 1], 0.0,
                        mybir.AluOpType.add, mybir.AluOpType.max)
                return h1sb

            def emit_L23(b, W, h1sb):
                h2sb = h2pool.tile([128, KO2, blk], BF16, tag="h2")
                for m in range(M1):
                    ps = pspool.tile([128, blk], F32, tag="ps")
                    for k in range(KO2):
                        nc.tensor.matmul(
                            ps[:],
                            W["w2"](k)[:, m * 128:(m + 1) * 128],
                            h1sb[:, k, :],
                            start=(k == 0), stop=(k == KO2 - 1))
                    nc.scalar.activation(
                        h2sb[:, m, :], ps[:], relu, bias=W["b2"][:, m:m + 1])
                ps3 = pspool.tile([128, blk], F32, tag="ps")
                for k in range(KO2):
                    nc.tensor.matmul(
                        ps3[:C, :],
                        W["w3"][:, k, :],
                        h2sb[:, k, :],
                        start=(k == 0), stop=(k == KO2 - 1))
                osb = opool.tile([C, blk], F32, tag="o")
                nc.scalar.activation(
                    osb[:], ps3[:C, :], ident, bias=W["b3"][:, 0:1])
                nc.gpsimd.dma_start(out_d[b], osb[:])

            # Software pipeline, depth 2: L1 of blocks b+1/b+2 are
            # emitted before L2/L3 of block b, so weight-set DMAs and
            # ACT latency never drain the PE (esp. during the initial
            # HBM-bound weight load).
            Ws = {}
            h1 = {}

            xpre = {}

            def emit_front(b):
                g = runs[b]
                if g not in Ws:
                    Ws[g] = emit_weights(g)
                h1[b] = emit_L1(Ws[g], xpre.pop(b) if b in xpre
                                else emit_x(b))

            # Startup: x0/x1 lead the scalar ring while weights
            # stream on sync; L1(0)/L1(1) are emitted before L2(0) so
            # the PE has work during the HBM-bound weight load. Steady
            # state keeps L1 two blocks ahead of L2/L3.
            def emit_x_sync(b):
                xsb = xpool.tile([128, KO1, blk], BF16, tag="x")
                nc.sync.dma_start(xsb[:], x_d[b])
                return xsb

            # Startup: everything for the first ~3 blocks rides the sync
            # ring in consumption order (x0 interleaved with W1 chunks,
            # then x1, W2a, x2, W2b); block 0's L1 runs k-outer so each
            # W1 chunk is consumed on arrival.
            g0 = runs[0]
            if prof[0] >= 3:
                xsb0 = xpool.tile([128, KO1, blk], BF16, tag="x")
                xs = {}
                def _x12():
                    xs[1] = emit_x_sync(1)
                    xs[2] = emit_x_sync(2)

                Ws[g0] = emit_weights(g0, x_first=(xsb0, x_d[0]),
                                      x_hook=_x12)
                # x3/x4 ride the idle SWDGE ring: the scalar ring's
                # issue slot is blocked behind early L2-relus right at
                # the prologue->steady transition.
                for bb in (3, 4):
                    if bb < NB:
                        xp = xpool.tile([128, KO1, blk], BF16, tag="x",
                                        name=f"xpre{bb}")
                        nc.gpsimd.dma_start(xp[:], x_d[bb])
                        xpre[bb] = xp
                h1[0] = emit_L1(Ws[g0], xsb0, kouter=True)
                h1[1] = emit_L1(Ws[g0], xs[1])
                h1[2] = emit_L1(Ws[g0], xs[2])
                emitted = 2
            elif NB > 1 and runs[1] == g0:
                xsb0 = xpool.tile([128, KO1, blk], BF16, tag="x")
                xs1 = []
                Ws[g0] = emit_weights(g0, x_first=(xsb0, x_d[0]),
                                      x_hook=lambda: xs1.append(emit_x(1)))
                h1[0] = emit_L1(Ws[g0], xsb0, kouter=True)
                h1[1] = emit_L1(Ws[g0], xs1[0])
                emitted = 1
            else:
                emit_front(0)
                emitted = 0
            for b in range(NB):
                for nxt in range(emitted + 1, min(b + 3, NB)):
                    emit_front(nxt)
                    emitted = nxt
                if b + 4 < NB and runs[b + 4] not in Ws:
                    Ws[runs[b + 4]] = emit_weights(runs[b + 4])
                emit_L23(b, Ws[runs[b]], h1.pop(b))

    nc.compile()
    _program_cache[key] = nc
    return nc


# ---------------------------------------------------------------- host
def _execute(inputs, trace=False, trace_cores=None):
    graph = np.ascontiguousarray(inputs["graph"], dtype=np.float32)
    state = np.ascontiguousarray(inputs["state"], dtype=np.float32)
    next_state = np.ascontiguousarray(inputs["next_state"], dtype=np.float32)
    W1 = np.ascontiguousarray(inputs["W1"], dtype=np.float32)
    b1 = np.ascontiguousarray(inputs["b1"], dtype=np.float32)
    W2 = np.ascontiguousarray(inputs["W2"], dtype=np.float32)
    b2 = np.ascontiguousarray(inputs["b2"], dtype=np.float32)
    W3 = np.ascontiguousarray(inputs["W3"], dtype=np.float32)
    b3 = np.ascontiguousarray(inputs["b3"], dtype=np.float32)

    B = graph.shape[0]
    NF, IN, H = W1.shape
    C = W3.shape[2]
    assert IN == graph.shape[1] + state.shape[1] + next_state.shape[1]
    assert H % 128 == 0 and C <= 128
    INP = ((IN + 127) // 128) * 128
    KO1 = INP // 128

    out_full = np.zeros((B, C), dtype=np.float32)

    # --- route: last active factor per row
    mask = graph[:, :NF] == 1.0
    active = mask.any(axis=1)
    last = (NF - 1) - np.argmax(mask[:, ::-1], axis=1)
    if not active.any():
        return (out_full, None) if trace else out_full

    rows_by_e = [np.nonzero(active & (last == e))[0] for e in range(NF)]
    nblk = [(len(r) + BLK - 1) // BLK for r in rows_by_e]
    prof, expert_of = _make_plan(nblk)
    G, NB = len(prof), sum(prof)

    # --- pack rows into per-core block slots
    # rowmap[core] : int32 [NB, BLK], original row id or -1 (pad)
    rowmap = [np.full((NB, BLK), -1, dtype=np.int64) for _ in range(NCORES)]
    off = np.cumsum([0] + prof)  # run g occupies blocks [off[g], off[g+1])
    slots_by_e = {}
    for core in range(NCORES):
        for g in range(G):
            slots_by_e.setdefault(expert_of[core][g], []).append((core, g))
    for e in range(NF):
        rows = rows_by_e[e]
        if len(rows) == 0:
            continue
        pos = 0
        for core, g in slots_by_e.get(e, []):
            cap = prof[g] * BLK
            take = min(cap, len(rows) - pos)
            if take <= 0:
                break
            flat = rowmap[core][off[g]:off[g + 1]].reshape(-1)
            flat[:take] = rows[pos:pos + take]
            pos += take
        assert pos == len(rows), f"expert {e} rows not fully packed"

    # --- build per-core inputs
    x = np.concatenate([graph, state, next_state], axis=1)  # [B, IN]
    if INP != IN:
        x = np.concatenate([x, np.zeros((B, INP - IN), np.float32)], axis=1)
    xpad = np.concatenate([x, np.zeros((1, INP), np.float32)], axis=0)
    W1p = np.zeros((NF, INP, H), np.float32)
    W1p[:, :IN] = W1

    # Partition-major device layouts: [.., 128, KO, free] so every DMA
    # line is one contiguous 10-20KB run per partition.
    KO2 = H // 128
    W1pm = np.ascontiguousarray(
        W1p.reshape(NF, KO1, 128, H).transpose(0, 2, 1, 3)).astype(bfloat16)
    W2pm = np.ascontiguousarray(
        W2.reshape(NF, KO2, 128, H).transpose(0, 2, 1, 3)).astype(bfloat16)
    W3pm = np.ascontiguousarray(
        W3.reshape(NF, KO2, 128, C).transpose(0, 2, 1, 3)).astype(bfloat16)
    in_maps = []
    for core in range(NCORES):
        xb = xpad[rowmap[core].reshape(-1)]  # [NB*BLK, INP]; -1 -> zero row
        xb = np.ascontiguousarray(
            xb.reshape(NB, BLK, KO1, 128).transpose(0, 3, 2, 1)).astype(bfloat16)
        es = expert_of[core]
        in_maps.append({
            "xb": xb,
            "w1": W1pm[es],
            "w2": W2pm[es],
            "w3": W3pm[es],
            "b1": np.ascontiguousarray(b1[es]),
            "b2": np.ascontiguousarray(b2[es]),
            "b3": np.ascontiguousarray(b3[es]),
        })

    nc = _build_program(prof, KO1, KO2, H, C, BLK)
    kwargs = {}
    if trace:
        kwargs = dict(trace=True,
                      trace_cores=trace_cores or list(range(NCORES)))
    res = run_bass_kernel_spmd(nc, in_maps, list(range(NCORES)), **kwargs)

    # --- scatter back
    for core in range(NCORES):
        ob = np.asarray(res.results[core]["outb"])  # [NB, C, BLK]
        rows = ob.transpose(0, 2, 1).reshape(NB * BLK, C)
        ids = rowmap[core].reshape(-1)
        valid = ids >= 0
        out_full[ids[valid]] = rows[valid]

    return (out_full, res) if trace else out_full


def kernel(**inputs):
    return _execute(inputs)

